# revision 11
# baseline (speedup 1.0000x reference)
"""Multi-head attention (B=2, N=2048, D=1024, H=16) on 8 Trainium2 cores.

Sharding: data-parallel over batch (cores 0-3 -> b=0, cores 4-7 -> b=1) and
tensor-parallel over heads (4 heads per core, i.e. 256 of the 1024 QKV/O
channels).  Each core computes its 4 heads' attention plus a partial output
projection; the host sums the 4 partials per batch and adds bo.

Projections and output projection run in float32r (fp32 data, full-rate PE
mode).  The attention matmuls (QK^T scores and PV) run in bf16 operands with
fp32 PSUM accumulation: f32r matmuls self-load their 4-byte stationary
operand (~430ns serial per matmul), which starves the PE; bf16 weight loads
hide completely.
"""

import numpy as np

import concourse.bass as bass
import concourse.bacc as bacc
import concourse.tile as tile
from concourse import mybir
from concourse.bass_utils import run_bass_kernel_spmd

F32 = mybir.dt.float32
F32R = mybir.dt.float32r
BF16 = mybir.dt.bfloat16
AF = mybir.ActivationFunctionType

B, N, D, H, HD = 2, 2048, 1024, 16, 64
E = 256            # channels per core (4 heads * 64)
DC = D // 128      # 8 contraction chunks for projections
NB = N // 128      # 16 token blocks / k chunks
SCALE = 1.0 / np.sqrt(HD)
DT_SC = F32R       # dtype for scores matmul operands (qt/kt)
DT_PV = BF16       # dtype for PV matmul operands (vp, w=exp out)


def _emit(nc):
    xT = nc.dram_tensor("xT", [D, N], F32R, kind="ExternalInput")
    wqT = nc.dram_tensor("wqT", [D, E], F32R, kind="ExternalInput")
    wkT = nc.dram_tensor("wkT", [D, E], F32R, kind="ExternalInput")
    wvT = nc.dram_tensor("wvT", [D, E], F32R, kind="ExternalInput")
    woT = nc.dram_tensor("woT", [E, D], F32R, kind="ExternalInput")
    bq2 = nc.dram_tensor("bq2", [128, 2], F32, kind="ExternalInput")
    bk2 = nc.dram_tensor("bk2", [128, 2], F32, kind="ExternalInput")
    bv1 = nc.dram_tensor("bv1", [E], F32, kind="ExternalInput")
    vones = nc.dram_tensor("vones", [128, NB, 4], DT_PV, kind="ExternalInput")
    out = nc.dram_tensor("out", [N, D], F32, kind="ExternalOutput")

    with tile.TileContext(nc) as tc:
        with tc.tile_pool(name="per", bufs=1) as per, \
             tc.tile_pool(name="wp", bufs=1) as wp, \
             tc.tile_pool(name="dn", bufs=1) as dn, \
             tc.tile_pool(name="up", bufs=2) as up, \
             tc.tile_pool(name="op", bufs=2) as op, \
             tc.tile_pool(name="ps", bufs=1, space="PSUM") as ps:

            # ---- persistent SBUF tiles ----
            xt = per.tile([128, DC, N], F32R)          # x[b].T  (d-chunk, tokens)
            wq = per.tile([128, DC, E], F32R)
            wk = per.tile([128, DC, E], F32R)
            wv = per.tile([128, DC, E], F32R)
            wo = per.tile([128, 2, D], F32R)           # WoT (e-chunk)
            qt = per.tile([128, 2, N], DT_SC)         # Q^T packed: pair, head-half
            kt = per.tile([128, 2, N], DT_SC)
            vp = per.tile([128, NB, 4, HD + 4], DT_PV)  # V + ones col, padded stride
            at = per.tile([128, 2, N], F32R)           # attn^T normalized
            bqs = per.tile([128, 2], F32)
            bks = per.tile([128, 2], F32)
            bvb = per.tile([128, E], F32)              # bv broadcast across parts
            ones = per.tile([1, HD], F32)

            for dc in range(DC):
                nc.sync.dma_start(out=xt[:, dc, :], in_=xT[dc * 128:(dc + 1) * 128, :])
                nc.sync.dma_start(out=wq[:, dc, :], in_=wqT[dc * 128:(dc + 1) * 128, :])
                nc.sync.dma_start(out=wk[:, dc, :], in_=wkT[dc * 128:(dc + 1) * 128, :])
                nc.sync.dma_start(out=wv[:, dc, :], in_=wvT[dc * 128:(dc + 1) * 128, :])
            for ec in range(2):
                nc.sync.dma_start(out=wo[:, ec, :], in_=woT[ec * 128:(ec + 1) * 128, :])
            nc.sync.dma_start(out=bqs, in_=bq2[:, :])
            nc.sync.dma_start(out=bks, in_=bk2[:, :])
            bv_ap = bv1[:]
            nc.gpsimd.dma_start(
                out=bvb,
                in_=bass.AP(tensor=bv_ap.tensor, offset=0, ap=[[0, 128], [1, E]]),
            )
            nc.vector.memset(ones, 1.0)
            nc.sync.dma_start(out=vp[:, :, :, HD:HD + 1],
                              in_=vones[:, :, :].rearrange("p a (b o) -> p a b o", o=1))

            # ---- phase 1: projections ----
            # V natural: psum[n(128) x e(256)] accumulated over 8 d-chunks
            for nb in range(NB):
                tag = "pv1" if nb % 2 == 0 else "pv2"
                pt = ps.tile([128, E], F32, tag=tag, name=f"psv{nb}")
                for dc in range(DC):
                    nc.tensor.matmul(
                        pt[:, :],
                        xt[:, dc, nb * 128:(nb + 1) * 128],
                        wv[:, dc, :],
                        start=(dc == 0), stop=(dc == DC - 1),
                    )
                nc.vector.tensor_add(
                    vp[:, nb, :, 0:HD],
                    pt.rearrange("p (h d) -> p h d", h=4),
                    bvb.rearrange("p (h d) -> p h d", h=4),
                )
            # Q^T / K^T: psum[e(128) x n(512)] accumulated over 8 d-chunks
            for pair in range(2):
                for wsb, dst, bias in ((wq, qt, bqs), (wk, kt, bks)):
                    for n4 in range(4):
                        tag = "s1" if n4 % 2 == 0 else "s2"
                        pt = ps.tile([128, 512], F32, tag=tag, name=f"psp{pair}{n4}")
                        for dc in range(DC):
                            nc.tensor.matmul(
                                pt[:, :],
                                wsb[:, dc, pair * 128:(pair + 1) * 128],
                                xt[:, dc, n4 * 512:(n4 + 1) * 512],
                                start=(dc == 0), stop=(dc == DC - 1),
                            )
                        nc.scalar.activation(
                            dst[:, pair, n4 * 512:(n4 + 1) * 512], pt[:, :],
                            AF.Identity, bias=bias[:, pair:pair + 1],
                        )

            # ---- phase 2+3 interleaved by q-half for cross-phase overlap ----
            for q2 in range(2):
                q0 = q2 * 1024
                for pair in range(2):
                    pv = [ps.tile([HD + 1, 1024], F32, tag=t, name=f"pv_{t}")
                          for t in ("pv1", "pv2")]
                    for k in range(NB):
                        stiles = [ps.tile([128, 1024], F32, tag=("s1", "s2")[hh],
                                          name=f"st{hh}") for hh in range(2)]
                        # interleave head halves so row-tiled matmuls overlap
                        for half in range(2):
                            for hh in range(2):
                                p0 = hh * 64
                                nc.tensor.matmul(
                                    stiles[hh][:, half * 512:(half + 1) * 512],
                                    kt[p0:p0 + 64, pair, k * 128:(k + 1) * 128],
                                    qt[p0:p0 + 64, pair,
                                       q0 + half * 512:q0 + (half + 1) * 512],
                                    start=True, stop=True,
                                    tile_position=(p0, 0),
                                )
                        for hh in range(2):
                            w = wp.tile([128, 1024], DT_PV, tag=("w1", "w2")[hh],
                                        name=f"w{hh}")
                            nc.scalar.activation(w, stiles[hh], AF.Exp, scale=SCALE)
                            for half in range(2):
                                nc.tensor.matmul(
                                    pv[hh][:, half * 512:(half + 1) * 512],
                                    vp[:, k, 2 * pair + hh, 0:HD + 1],
                                    w[:, half * 512:(half + 1) * 512],
                                    start=(k == 0), stop=(k == NB - 1),
                                )
                    # normalize: attn^T[d, q] / den[q]
                    for hh in range(2):
                        den = dn.tile([1, 1024], F32, tag="den", name="den")
                        rec = dn.tile([1, 1024], F32, tag="rec", name="rec")
                        nc.vector.tensor_copy(den, pv[hh][HD:HD + 1, :])
                        nc.vector.reciprocal_approx_fast(rec, den)
                        bc = ps.tile([HD, 1024], F32, tag=("s1", "s2")[hh], name="bc")
                        for half in range(2):
                            nc.tensor.matmul(
                                bc[:, half * 512:(half + 1) * 512],
                                ones[:, :],
                                rec[:, half * 512:(half + 1) * 512],
                                start=True, stop=True,
                            )
                        u = up.tile([HD, 1024], F32)
                        nc.vector.tensor_copy(u, pv[hh][0:HD, :])
                        nc.vector.tensor_mul(
                            at[hh * 64:hh * 64 + 64, pair, q0:q0 + 1024], u, bc)

                # ---- output projection for this q-half ----
                for nb in range(q2 * 8, q2 * 8 + 8):
                    tag = "pv1" if nb % 2 == 0 else "pv2"
                    po = ps.tile([128, 1024], F32, tag=tag, name=f"po{nb}")
                    for half in range(2):
                        for ec in range(2):
                            nc.tensor.matmul(
                                po[:, half * 512:(half + 1) * 512],
                                at[:, ec, nb * 128:(nb + 1) * 128],
                                wo[:, ec, half * 512:(half + 1) * 512],
                                start=(ec == 0), stop=(ec == 1),
                            )
                    ot = op.tile([128, 1024], F32)
                    nc.scalar.activation(ot, po, AF.Copy)
                    nc.sync.dma_start(out=out[nb * 128:(nb + 1) * 128, :], in_=ot)
    return nc


_CACHE = {}


def _build():
    if "nc" not in _CACHE:
        nc = bacc.Bacc("TRN2", target_bir_lowering=False, debug=False)
        _emit(nc)
        nc.compile()
        _CACHE["nc"] = nc
    return _CACHE["nc"]


def make_in_maps(x, Wq, bq, Wk, bk, Wv, bv, Wo, bo):
    import ml_dtypes
    f32 = np.float32
    ones_np = np.ones((128, NB, 4),
                      ml_dtypes.bfloat16 if DT_PV == BF16 else f32)
    xTs = [np.ascontiguousarray(np.asarray(x[b], dtype=f32).T) for b in range(B)]
    in_maps = []
    for c in range(8):
        b, r0 = c // 4, (c % 4) * E
        rows = slice(r0, r0 + E)
        in_maps.append({
            "xT": xTs[b],
            "wqT": np.ascontiguousarray(np.asarray(Wq, f32)[rows].T),
            "wkT": np.ascontiguousarray(np.asarray(Wk, f32)[rows].T),
            "wvT": np.ascontiguousarray(np.asarray(Wv, f32)[rows].T),
            "woT": np.ascontiguousarray(np.asarray(Wo, f32)[:, rows].T),
            "bq2": np.ascontiguousarray(np.asarray(bq, f32)[rows].reshape(2, 128).T),
            "bk2": np.ascontiguousarray(np.asarray(bk, f32)[rows].reshape(2, 128).T),
            "bv1": np.ascontiguousarray(np.asarray(bv, f32)[rows]),
            "vones": ones_np,
        })
    return in_maps


def kernel(x, Wq, bq, Wk, bk, Wv, bv, Wo, bo, _spmd_kwargs=None):
    nc = _build()
    in_maps = make_in_maps(x, Wq, bq, Wk, bk, Wv, bv, Wo, bo)
    res = run_bass_kernel_spmd(nc, in_maps, core_ids=list(range(8)),
                               **(_spmd_kwargs or {}))
    parts = np.stack([res.results[c]["out"] for c in range(8)])
    outv = parts.reshape(B, 4, N, D).sum(axis=1) + np.asarray(bo, np.float32)
    if _spmd_kwargs:
        _CACHE["last_results"] = res
    return outv.astype(np.float32)


# revision 12
# speedup vs baseline: 1.1693x; 1.1693x over previous
"""Multi-head attention (B=2, N=2048, D=1024, H=16) on 8 Trainium2 cores.

Sharding: data-parallel over batch (cores 0-3 -> b=0, cores 4-7 -> b=1) and
tensor-parallel over heads (4 heads per core, i.e. 256 of the 1024 QKV/O
channels).  Each core computes its 4 heads' attention plus a partial output
projection; the host sums the 4 partials per batch and adds bo.

Projections and output projection run in float32r (fp32 data, full-rate PE
mode).  The attention matmuls (QK^T scores and PV) run in bf16 operands with
fp32 PSUM accumulation: f32r matmuls self-load their 4-byte stationary
operand (~430ns serial per matmul), which starves the PE; bf16 weight loads
hide completely.
"""

import numpy as np

import concourse.bass as bass
import concourse.bacc as bacc
import concourse.tile as tile
from concourse import mybir
from concourse.bass_utils import run_bass_kernel_spmd

F32 = mybir.dt.float32
F32R = mybir.dt.float32r
BF16 = mybir.dt.bfloat16
AF = mybir.ActivationFunctionType

B, N, D, H, HD = 2, 2048, 1024, 16, 64
E = 256            # channels per core (4 heads * 64)
DC = D // 128      # 8 contraction chunks for projections
NB = N // 128      # 16 token blocks / k chunks
SCALE = 1.0 / np.sqrt(HD)
DT_SC = BF16       # dtype for scores matmul operands (qt/kt)
DT_PV = BF16       # dtype for PV matmul operands (vp, w=exp out)


def _emit(nc):
    xT = nc.dram_tensor("xT", [D, N], F32R, kind="ExternalInput")
    wqT = nc.dram_tensor("wqT", [D, E], F32R, kind="ExternalInput")
    wkT = nc.dram_tensor("wkT", [D, E], F32R, kind="ExternalInput")
    wvT = nc.dram_tensor("wvT", [D, E], F32R, kind="ExternalInput")
    woT = nc.dram_tensor("woT", [E, D], F32R, kind="ExternalInput")
    bq2 = nc.dram_tensor("bq2", [128, 2], F32, kind="ExternalInput")
    bk2 = nc.dram_tensor("bk2", [128, 2], F32, kind="ExternalInput")
    bv1 = nc.dram_tensor("bv1", [E], F32, kind="ExternalInput")
    vones = nc.dram_tensor("vones", [128, NB, 4], DT_PV, kind="ExternalInput")
    out = nc.dram_tensor("out", [N, D], F32, kind="ExternalOutput")

    with tile.TileContext(nc) as tc:
        with tc.tile_pool(name="per", bufs=1) as per, \
             tc.tile_pool(name="wp", bufs=1) as wp, \
             tc.tile_pool(name="dn", bufs=1) as dn, \
             tc.tile_pool(name="up", bufs=2) as up, \
             tc.tile_pool(name="op", bufs=2) as op, \
             tc.tile_pool(name="ps", bufs=1, space="PSUM") as ps:

            # ---- persistent SBUF tiles ----
            xt = per.tile([128, DC, N], F32R)          # x[b].T  (d-chunk, tokens)
            wq = per.tile([128, DC, E], F32R)
            wk = per.tile([128, DC, E], F32R)
            wv = per.tile([128, DC, E], F32R)
            wo = per.tile([128, 2, D], F32R)           # WoT (e-chunk)
            qt = per.tile([128, 2, N], DT_SC)         # Q^T packed: pair, head-half
            kt = per.tile([128, 2, N], DT_SC)
            vp = per.tile([128, NB, 4, HD + 4], DT_PV)  # V + ones col, padded stride
            at = per.tile([128, 2, N], F32R)           # attn^T normalized
            bqs = per.tile([128, 2], F32)
            bks = per.tile([128, 2], F32)
            bvb = per.tile([128, E], F32)              # bv broadcast across parts
            ones = per.tile([1, HD], F32)

            for dc in range(DC):
                nc.sync.dma_start(out=xt[:, dc, :], in_=xT[dc * 128:(dc + 1) * 128, :])
                nc.sync.dma_start(out=wq[:, dc, :], in_=wqT[dc * 128:(dc + 1) * 128, :])
                nc.sync.dma_start(out=wk[:, dc, :], in_=wkT[dc * 128:(dc + 1) * 128, :])
                nc.sync.dma_start(out=wv[:, dc, :], in_=wvT[dc * 128:(dc + 1) * 128, :])
            for ec in range(2):
                nc.sync.dma_start(out=wo[:, ec, :], in_=woT[ec * 128:(ec + 1) * 128, :])
            nc.sync.dma_start(out=bqs, in_=bq2[:, :])
            nc.sync.dma_start(out=bks, in_=bk2[:, :])
            bv_ap = bv1[:]
            nc.gpsimd.dma_start(
                out=bvb,
                in_=bass.AP(tensor=bv_ap.tensor, offset=0, ap=[[0, 128], [1, E]]),
            )
            nc.vector.memset(ones, 1.0)
            nc.sync.dma_start(out=vp[:, :, :, HD:HD + 1],
                              in_=vones[:, :, :].rearrange("p a (b o) -> p a b o", o=1))

            # ---- phase 1: projections ----
            # V natural: psum[n(128) x e(256)] accumulated over 8 d-chunks
            for nb in range(NB):
                tag = "pv1" if nb % 2 == 0 else "pv2"
                pt = ps.tile([128, E], F32, tag=tag, name=f"psv{nb}")
                for dc in range(DC):
                    nc.tensor.matmul(
                        pt[:, :],
                        xt[:, dc, nb * 128:(nb + 1) * 128],
                        wv[:, dc, :],
                        start=(dc == 0), stop=(dc == DC - 1),
                    )
                nc.vector.tensor_add(
                    vp[:, nb, :, 0:HD],
                    pt.rearrange("p (h d) -> p h d", h=4),
                    bvb.rearrange("p (h d) -> p h d", h=4),
                )
            # Q^T / K^T: psum[e(128) x n(512)] accumulated over 8 d-chunks
            for pair in range(2):
                for wsb, dst, bias in ((wq, qt, bqs), (wk, kt, bks)):
                    for n4 in range(4):
                        tag = "s1" if n4 % 2 == 0 else "s2"
                        pt = ps.tile([128, 512], F32, tag=tag, name=f"psp{pair}{n4}")
                        for dc in range(DC):
                            nc.tensor.matmul(
                                pt[:, :],
                                wsb[:, dc, pair * 128:(pair + 1) * 128],
                                xt[:, dc, n4 * 512:(n4 + 1) * 512],
                                start=(dc == 0), stop=(dc == DC - 1),
                            )
                        nc.scalar.activation(
                            dst[:, pair, n4 * 512:(n4 + 1) * 512], pt[:, :],
                            AF.Identity, bias=bias[:, pair:pair + 1],
                        )

            # ---- phase 2+3 interleaved by q-half for cross-phase overlap ----
            for q2 in range(2):
                q0 = q2 * 1024
                for pair in range(2):
                    pv = [ps.tile([HD + 1, 1024], F32, tag=t, name=f"pv_{t}")
                          for t in ("pv1", "pv2")]
                    for k in range(NB):
                        stiles = [ps.tile([128, 1024], F32, tag=("s1", "s2")[hh],
                                          name=f"st{hh}") for hh in range(2)]
                        # interleave head halves so row-tiled matmuls overlap
                        for half in range(2):
                            for hh in range(2):
                                p0 = hh * 64
                                nc.tensor.matmul(
                                    stiles[hh][:, half * 512:(half + 1) * 512],
                                    kt[p0:p0 + 64, pair, k * 128:(k + 1) * 128],
                                    qt[p0:p0 + 64, pair,
                                       q0 + half * 512:q0 + (half + 1) * 512],
                                    start=True, stop=True,
                                    tile_position=(p0, 0),
                                )
                        for hh in range(2):
                            w = wp.tile([128, 1024], DT_PV, tag=("w1", "w2")[hh],
                                        name=f"w{hh}")
                            nc.scalar.activation(w, stiles[hh], AF.Exp, scale=SCALE)
                            for half in range(2):
                                nc.tensor.matmul(
                                    pv[hh][:, half * 512:(half + 1) * 512],
                                    vp[:, k, 2 * pair + hh, 0:HD + 1],
                                    w[:, half * 512:(half + 1) * 512],
                                    start=(k == 0), stop=(k == NB - 1),
                                )
                    # normalize: attn^T[d, q] / den[q]
                    for hh in range(2):
                        den = dn.tile([1, 1024], F32, tag="den", name="den")
                        rec = dn.tile([1, 1024], F32, tag="rec", name="rec")
                        nc.vector.tensor_copy(den, pv[hh][HD:HD + 1, :])
                        nc.vector.reciprocal_approx_fast(rec, den)
                        bc = ps.tile([HD, 1024], F32, tag=("s1", "s2")[hh], name="bc")
                        for half in range(2):
                            nc.tensor.matmul(
                                bc[:, half * 512:(half + 1) * 512],
                                ones[:, :],
                                rec[:, half * 512:(half + 1) * 512],
                                start=True, stop=True,
                            )
                        u = up.tile([HD, 1024], F32)
                        nc.vector.tensor_copy(u, pv[hh][0:HD, :])
                        nc.vector.tensor_mul(
                            at[hh * 64:hh * 64 + 64, pair, q0:q0 + 1024], u, bc)

                # ---- output projection for this q-half ----
                for nb in range(q2 * 8, q2 * 8 + 8):
                    tag = "pv1" if nb % 2 == 0 else "pv2"
                    po = ps.tile([128, 1024], F32, tag=tag, name=f"po{nb}")
                    for half in range(2):
                        for ec in range(2):
                            nc.tensor.matmul(
                                po[:, half * 512:(half + 1) * 512],
                                at[:, ec, nb * 128:(nb + 1) * 128],
                                wo[:, ec, half * 512:(half + 1) * 512],
                                start=(ec == 0), stop=(ec == 1),
                            )
                    ot = op.tile([128, 1024], F32)
                    nc.scalar.activation(ot, po, AF.Copy)
                    nc.sync.dma_start(out=out[nb * 128:(nb + 1) * 128, :], in_=ot)
    return nc


_CACHE = {}


def _build():
    if "nc" not in _CACHE:
        nc = bacc.Bacc("TRN2", target_bir_lowering=False, debug=False)
        _emit(nc)
        nc.compile()
        _CACHE["nc"] = nc
    return _CACHE["nc"]


def make_in_maps(x, Wq, bq, Wk, bk, Wv, bv, Wo, bo):
    import ml_dtypes
    f32 = np.float32
    ones_np = np.ones((128, NB, 4),
                      ml_dtypes.bfloat16 if DT_PV == BF16 else f32)
    xTs = [np.ascontiguousarray(np.asarray(x[b], dtype=f32).T) for b in range(B)]
    in_maps = []
    for c in range(8):
        b, r0 = c // 4, (c % 4) * E
        rows = slice(r0, r0 + E)
        in_maps.append({
            "xT": xTs[b],
            "wqT": np.ascontiguousarray(np.asarray(Wq, f32)[rows].T),
            "wkT": np.ascontiguousarray(np.asarray(Wk, f32)[rows].T),
            "wvT": np.ascontiguousarray(np.asarray(Wv, f32)[rows].T),
            "woT": np.ascontiguousarray(np.asarray(Wo, f32)[:, rows].T),
            "bq2": np.ascontiguousarray(np.asarray(bq, f32)[rows].reshape(2, 128).T),
            "bk2": np.ascontiguousarray(np.asarray(bk, f32)[rows].reshape(2, 128).T),
            "bv1": np.ascontiguousarray(np.asarray(bv, f32)[rows]),
            "vones": ones_np,
        })
    return in_maps


def kernel(x, Wq, bq, Wk, bk, Wv, bv, Wo, bo, _spmd_kwargs=None):
    nc = _build()
    in_maps = make_in_maps(x, Wq, bq, Wk, bk, Wv, bv, Wo, bo)
    res = run_bass_kernel_spmd(nc, in_maps, core_ids=list(range(8)),
                               **(_spmd_kwargs or {}))
    parts = np.stack([res.results[c]["out"] for c in range(8)])
    outv = parts.reshape(B, 4, N, D).sum(axis=1) + np.asarray(bo, np.float32)
    if _spmd_kwargs:
        _CACHE["last_results"] = res
    return outv.astype(np.float32)


# revision 13
# speedup vs baseline: 1.2620x; 1.0792x over previous
"""Multi-head attention (B=2, N=2048, D=1024, H=16) on 8 Trainium2 cores.

Sharding: data-parallel over batch (cores 0-3 -> b=0, cores 4-7 -> b=1) and
tensor-parallel over heads (4 heads per core, i.e. 256 of the 1024 QKV/O
channels).  Each core computes its 4 heads' attention plus a partial output
projection; the host sums the 4 partials per batch and adds bo.

Projections and output projection run in float32r (fp32 data, full-rate PE
mode).  The attention matmuls (QK^T scores and PV) run in bf16 operands with
fp32 PSUM accumulation: f32r matmuls self-load their 4-byte stationary
operand (~430ns serial per matmul), which starves the PE; bf16 weight loads
hide completely.
"""

import numpy as np

import concourse.bass as bass
import concourse.bacc as bacc
import concourse.tile as tile
from concourse import mybir
from concourse.bass_utils import run_bass_kernel_spmd

F32 = mybir.dt.float32
F32R = mybir.dt.float32r
BF16 = mybir.dt.bfloat16
AF = mybir.ActivationFunctionType

B, N, D, H, HD = 2, 2048, 1024, 16, 64
E = 256            # channels per core (4 heads * 64)
DC = D // 128      # 8 contraction chunks for projections
NB = N // 128      # 16 token blocks / k chunks
SCALE = 1.0 / np.sqrt(HD)
DT_PR = BF16       # dtype for projection matmul operands (x, Wq/Wk/Wv)
DT_SC = BF16       # dtype for scores matmul operands (qt/kt)
DT_PV = BF16       # dtype for PV matmul operands (vp, w=exp out)


def _emit(nc):
    xT = nc.dram_tensor("xT", [D, N], DT_PR, kind="ExternalInput")
    wqT = nc.dram_tensor("wqT", [D, E], DT_PR, kind="ExternalInput")
    wkT = nc.dram_tensor("wkT", [D, E], DT_PR, kind="ExternalInput")
    wvT = nc.dram_tensor("wvT", [D, E], DT_PR, kind="ExternalInput")
    woT = nc.dram_tensor("woT", [E, D], F32R, kind="ExternalInput")
    bq2 = nc.dram_tensor("bq2", [128, 2], F32, kind="ExternalInput")
    bk2 = nc.dram_tensor("bk2", [128, 2], F32, kind="ExternalInput")
    bv1 = nc.dram_tensor("bv1", [E], F32, kind="ExternalInput")
    vones = nc.dram_tensor("vones", [128, NB, 4], DT_PV, kind="ExternalInput")
    onesr = nc.dram_tensor("onesr", [1, HD], F32R, kind="ExternalInput")
    out = nc.dram_tensor("out", [N, D], F32, kind="ExternalOutput")

    with tile.TileContext(nc) as tc:
        with tc.tile_pool(name="per", bufs=1) as per, \
             tc.tile_pool(name="wp", bufs=1) as wp, \
             tc.tile_pool(name="dn", bufs=1) as dn, \
             tc.tile_pool(name="up", bufs=2) as up, \
             tc.tile_pool(name="op", bufs=2) as op, \
             tc.tile_pool(name="ps", bufs=1, space="PSUM") as ps:

            # ---- persistent SBUF tiles ----
            xt = per.tile([128, DC, N], DT_PR)          # x[b].T  (d-chunk, tokens)
            wq = per.tile([128, DC, E], DT_PR)
            wk = per.tile([128, DC, E], DT_PR)
            wv = per.tile([128, DC, E], DT_PR)
            wo = per.tile([128, 2, D], F32R)           # WoT (e-chunk)
            qt = per.tile([128, 2, N], DT_SC)         # Q^T packed: pair, head-half
            kt = per.tile([128, 2, N], DT_SC)
            vp = per.tile([128, NB, 4, HD + 4], DT_PV)  # V + ones col, padded stride
            at = per.tile([128, 2, N], F32R)           # attn^T normalized
            bqs = per.tile([128, 2], F32)
            bks = per.tile([128, 2], F32)
            bvb = per.tile([128, E], F32)              # bv broadcast across parts
            ones = per.tile([1, HD], F32R)

            for dc in range(DC):
                nc.sync.dma_start(out=xt[:, dc, :], in_=xT[dc * 128:(dc + 1) * 128, :])
                nc.sync.dma_start(out=wq[:, dc, :], in_=wqT[dc * 128:(dc + 1) * 128, :])
                nc.sync.dma_start(out=wk[:, dc, :], in_=wkT[dc * 128:(dc + 1) * 128, :])
                nc.sync.dma_start(out=wv[:, dc, :], in_=wvT[dc * 128:(dc + 1) * 128, :])
            for ec in range(2):
                nc.sync.dma_start(out=wo[:, ec, :], in_=woT[ec * 128:(ec + 1) * 128, :])
            nc.sync.dma_start(out=bqs, in_=bq2[:, :])
            nc.sync.dma_start(out=bks, in_=bk2[:, :])
            bv_ap = bv1[:]
            nc.gpsimd.dma_start(
                out=bvb,
                in_=bass.AP(tensor=bv_ap.tensor, offset=0, ap=[[0, 128], [1, E]]),
            )
            nc.sync.dma_start(out=ones, in_=onesr[:, :])
            nc.sync.dma_start(out=vp[:, :, :, HD:HD + 1],
                              in_=vones[:, :, :].rearrange("p a (b o) -> p a b o", o=1))

            # ---- phase 1: projections ----
            # V natural: psum[n(128) x e(256)] accumulated over 8 d-chunks
            for nb in range(NB):
                tag = "pv1" if nb % 2 == 0 else "pv2"
                pt = ps.tile([128, E], F32, tag=tag, name=f"psv{nb}")
                for dc in range(DC):
                    nc.tensor.matmul(
                        pt[:, :],
                        xt[:, dc, nb * 128:(nb + 1) * 128],
                        wv[:, dc, :],
                        start=(dc == 0), stop=(dc == DC - 1),
                    )
                nc.vector.tensor_add(
                    vp[:, nb, :, 0:HD],
                    pt.rearrange("p (h d) -> p h d", h=4),
                    bvb.rearrange("p (h d) -> p h d", h=4),
                )
            # Q^T / K^T: psum[e(128) x n(512)] accumulated over 8 d-chunks
            for pair in range(2):
                for wsb, dst, bias in ((wq, qt, bqs), (wk, kt, bks)):
                    for n4 in range(4):
                        tag = "s1" if n4 % 2 == 0 else "s2"
                        pt = ps.tile([128, 512], F32, tag=tag, name=f"psp{pair}{n4}")
                        for dc in range(DC):
                            nc.tensor.matmul(
                                pt[:, :],
                                wsb[:, dc, pair * 128:(pair + 1) * 128],
                                xt[:, dc, n4 * 512:(n4 + 1) * 512],
                                start=(dc == 0), stop=(dc == DC - 1),
                            )
                        nc.vector.tensor_scalar_add(
                            dst[:, pair, n4 * 512:(n4 + 1) * 512], pt[:, :],
                            bias[:, pair:pair + 1],
                        )

            # ---- phase 2+3 interleaved by q-half for cross-phase overlap ----
            for q2 in range(2):
                q0 = q2 * 1024
                for pair in range(2):
                    pv = [ps.tile([HD + 1, 1024], F32, tag=t, name=f"pv_{t}")
                          for t in ("pv1", "pv2")]
                    for k in range(NB):
                        stiles = [ps.tile([128, 1024], F32, tag=("s1", "s2")[hh],
                                          name=f"st{hh}") for hh in range(2)]
                        # interleave head halves so row-tiled matmuls overlap
                        for half in range(2):
                            for hh in range(2):
                                p0 = hh * 64
                                nc.tensor.matmul(
                                    stiles[hh][:, half * 512:(half + 1) * 512],
                                    kt[p0:p0 + 64, pair, k * 128:(k + 1) * 128],
                                    qt[p0:p0 + 64, pair,
                                       q0 + half * 512:q0 + (half + 1) * 512],
                                    start=True, stop=True,
                                    tile_position=(p0, 0),
                                )
                        for hh in range(2):
                            w = wp.tile([128, 1024], DT_PV, tag=("w1", "w2")[hh],
                                        name=f"w{hh}")
                            nc.scalar.activation(w, stiles[hh], AF.Exp, scale=SCALE)
                            for half in range(2):
                                nc.tensor.matmul(
                                    pv[hh][:, half * 512:(half + 1) * 512],
                                    vp[:, k, 2 * pair + hh, 0:HD + 1],
                                    w[:, half * 512:(half + 1) * 512],
                                    start=(k == 0), stop=(k == NB - 1),
                                )
                    # normalize: attn^T[d, q] / den[q]
                    for hh in range(2):
                        den = dn.tile([1, 1024], F32, tag="den", name="den")
                        rec = dn.tile([1, 1024], F32, tag="rec", name="rec")
                        recr = dn.tile([1, 1024], F32R, tag="recr", name="recr")
                        nc.vector.tensor_copy(den, pv[hh][HD:HD + 1, :])
                        nc.vector.reciprocal_approx_fast(rec, den)
                        nc.vector.tensor_copy(recr, rec)
                        bc = ps.tile([HD, 1024], F32, tag=("s1", "s2")[hh], name="bc")
                        for half in range(2):
                            nc.tensor.matmul(
                                bc[:, half * 512:(half + 1) * 512],
                                ones[:, :],
                                recr[:, half * 512:(half + 1) * 512],
                                start=True, stop=True,
                            )
                        u = up.tile([HD, 1024], F32)
                        nc.vector.tensor_copy(u, pv[hh][0:HD, :])
                        nc.vector.tensor_mul(
                            at[hh * 64:hh * 64 + 64, pair, q0:q0 + 1024], u, bc)

                # ---- output projection for this q-half ----
                for nb in range(q2 * 8, q2 * 8 + 8):
                    tag = "pv1" if nb % 2 == 0 else "pv2"
                    po = ps.tile([128, 1024], F32, tag=tag, name=f"po{nb}")
                    for half in range(2):
                        for ec in range(2):
                            nc.tensor.matmul(
                                po[:, half * 512:(half + 1) * 512],
                                at[:, ec, nb * 128:(nb + 1) * 128],
                                wo[:, ec, half * 512:(half + 1) * 512],
                                start=(ec == 0), stop=(ec == 1),
                            )
                    ot = op.tile([128, 1024], F32)
                    nc.vector.tensor_copy(ot, po)
                    nc.sync.dma_start(out=out[nb * 128:(nb + 1) * 128, :], in_=ot)
    return nc


_CACHE = {}


def _build():
    if "nc" not in _CACHE:
        nc = bacc.Bacc("TRN2", target_bir_lowering=False, debug=False)
        _emit(nc)
        nc.compile()
        _CACHE["nc"] = nc
    return _CACHE["nc"]


def make_in_maps(x, Wq, bq, Wk, bk, Wv, bv, Wo, bo):
    import ml_dtypes
    f32 = np.float32
    bt = ml_dtypes.bfloat16
    dpr = bt if DT_PR == BF16 else f32
    ones_np = np.ones((128, NB, 4), bt if DT_PV == BF16 else f32)
    xTs = [np.ascontiguousarray(np.asarray(x[b], dtype=f32).T).astype(dpr)
           for b in range(B)]
    in_maps = []
    for c in range(8):
        b, r0 = c // 4, (c % 4) * E
        rows = slice(r0, r0 + E)
        in_maps.append({
            "xT": xTs[b],
            "wqT": np.ascontiguousarray(np.asarray(Wq, f32)[rows].T).astype(dpr),
            "wkT": np.ascontiguousarray(np.asarray(Wk, f32)[rows].T).astype(dpr),
            "wvT": np.ascontiguousarray(np.asarray(Wv, f32)[rows].T).astype(dpr),
            "woT": np.ascontiguousarray(np.asarray(Wo, f32)[:, rows].T),
            "bq2": np.ascontiguousarray(np.asarray(bq, f32)[rows].reshape(2, 128).T),
            "bk2": np.ascontiguousarray(np.asarray(bk, f32)[rows].reshape(2, 128).T),
            "bv1": np.ascontiguousarray(np.asarray(bv, f32)[rows]),
            "vones": ones_np,
            "onesr": np.ones((1, HD), f32),
        })
    return in_maps


def kernel(x, Wq, bq, Wk, bk, Wv, bv, Wo, bo, _spmd_kwargs=None):
    nc = _build()
    in_maps = make_in_maps(x, Wq, bq, Wk, bk, Wv, bv, Wo, bo)
    res = run_bass_kernel_spmd(nc, in_maps, core_ids=list(range(8)),
                               **(_spmd_kwargs or {}))
    parts = np.stack([res.results[c]["out"] for c in range(8)])
    outv = parts.reshape(B, 4, N, D).sum(axis=1) + np.asarray(bo, np.float32)
    if _spmd_kwargs:
        _CACHE["last_results"] = res
    return outv.astype(np.float32)


# revision 14
# speedup vs baseline: 1.3456x; 1.0662x over previous
"""Multi-head attention (B=2, N=2048, D=1024, H=16) on 8 Trainium2 cores.

Sharding: data-parallel over batch (cores 0-3 -> b=0, cores 4-7 -> b=1) and
tensor-parallel over heads (4 heads per core, i.e. 256 of the 1024 QKV/O
channels).  Each core computes its 4 heads' attention plus a partial output
projection; the host sums the 4 partials per batch and adds bo.

Projections and output projection run in float32r (fp32 data, full-rate PE
mode).  The attention matmuls (QK^T scores and PV) run in bf16 operands with
fp32 PSUM accumulation: f32r matmuls self-load their 4-byte stationary
operand (~430ns serial per matmul), which starves the PE; bf16 weight loads
hide completely.
"""

import numpy as np

import concourse.bass as bass
import concourse.bacc as bacc
import concourse.tile as tile
from concourse import mybir
from concourse.bass_utils import run_bass_kernel_spmd

F32 = mybir.dt.float32
F32R = mybir.dt.float32r
BF16 = mybir.dt.bfloat16
AF = mybir.ActivationFunctionType

B, N, D, H, HD = 2, 2048, 1024, 16, 64
E = 256            # channels per core (4 heads * 64)
DC = D // 128      # 8 contraction chunks for projections
NB = N // 128      # 16 token blocks / k chunks
SCALE = 1.0 / np.sqrt(HD)
DT_PR = BF16       # dtype for projection matmul operands (x, Wq/Wk/Wv)
DT_SC = BF16       # dtype for scores matmul operands (qt/kt)
DT_PV = BF16       # dtype for PV matmul operands (vp, w=exp out)


def _emit(nc):
    xT = nc.dram_tensor("xT", [D, N], DT_PR, kind="ExternalInput")
    wqT = nc.dram_tensor("wqT", [D, E], DT_PR, kind="ExternalInput")
    wkT = nc.dram_tensor("wkT", [D, E], DT_PR, kind="ExternalInput")
    wvT = nc.dram_tensor("wvT", [D, E], DT_PR, kind="ExternalInput")
    woT = nc.dram_tensor("woT", [E, D], F32R, kind="ExternalInput")
    bq2 = nc.dram_tensor("bq2", [128, 2], F32, kind="ExternalInput")
    bk2 = nc.dram_tensor("bk2", [128, 2], F32, kind="ExternalInput")
    bv1 = nc.dram_tensor("bv1", [E], F32, kind="ExternalInput")
    vones = nc.dram_tensor("vones", [128, NB, 4], DT_PV, kind="ExternalInput")
    onesr = nc.dram_tensor("onesr", [1, HD], F32R, kind="ExternalInput")
    out = nc.dram_tensor("out", [N, D], F32, kind="ExternalOutput")

    with tile.TileContext(nc) as tc:
        with tc.tile_pool(name="per", bufs=1) as per, \
             tc.tile_pool(name="wp", bufs=2) as wp, \
             tc.tile_pool(name="dn", bufs=2) as dn, \
             tc.tile_pool(name="up", bufs=2) as up, \
             tc.tile_pool(name="op", bufs=2) as op, \
             tc.tile_pool(name="ps", bufs=1, space="PSUM") as ps:

            # ---- persistent SBUF tiles ----
            xt = per.tile([128, DC, N], DT_PR)          # x[b].T  (d-chunk, tokens)
            wq = per.tile([128, DC, E], DT_PR)
            wk = per.tile([128, DC, E], DT_PR)
            wv = per.tile([128, DC, E], DT_PR)
            wo = per.tile([128, 2, D], F32R)           # WoT (e-chunk)
            qt = per.tile([128, 2, N], DT_SC)         # Q^T packed: pair, head-half
            kt = per.tile([128, 2, N], DT_SC)
            vp = per.tile([128, NB, 4, HD + 4], DT_PV)  # V + ones col, padded stride
            at = per.tile([128, 2, N], F32R)           # attn^T normalized
            bqs = per.tile([128, 2], F32)
            bks = per.tile([128, 2], F32)
            bvb = per.tile([128, E], F32)              # bv broadcast across parts
            ones = per.tile([1, HD], F32R)

            for dc in range(DC):
                nc.sync.dma_start(out=xt[:, dc, :], in_=xT[dc * 128:(dc + 1) * 128, :])
                nc.sync.dma_start(out=wq[:, dc, :], in_=wqT[dc * 128:(dc + 1) * 128, :])
                nc.sync.dma_start(out=wk[:, dc, :], in_=wkT[dc * 128:(dc + 1) * 128, :])
                nc.sync.dma_start(out=wv[:, dc, :], in_=wvT[dc * 128:(dc + 1) * 128, :])
            for ec in range(2):
                nc.sync.dma_start(out=wo[:, ec, :], in_=woT[ec * 128:(ec + 1) * 128, :])
            nc.sync.dma_start(out=bqs, in_=bq2[:, :])
            nc.sync.dma_start(out=bks, in_=bk2[:, :])
            bv_ap = bv1[:]
            nc.gpsimd.dma_start(
                out=bvb,
                in_=bass.AP(tensor=bv_ap.tensor, offset=0, ap=[[0, 128], [1, E]]),
            )
            nc.sync.dma_start(out=ones, in_=onesr[:, :])
            nc.sync.dma_start(out=vp[:, :, :, HD:HD + 1],
                              in_=vones[:, :, :].rearrange("p a (b o) -> p a b o", o=1))

            # ---- phase 1: projections ----
            # V natural: psum[n(128) x e(256)] accumulated over 8 d-chunks
            for nb in range(NB):
                tag = "pv1" if nb % 2 == 0 else "pv2"
                pt = ps.tile([128, E], F32, tag=tag, name=f"psv{nb}")
                for dc in range(DC):
                    nc.tensor.matmul(
                        pt[:, :],
                        xt[:, dc, nb * 128:(nb + 1) * 128],
                        wv[:, dc, :],
                        start=(dc == 0), stop=(dc == DC - 1),
                    )
                nc.vector.tensor_add(
                    vp[:, nb, :, 0:HD],
                    pt.rearrange("p (h d) -> p h d", h=4),
                    bvb.rearrange("p (h d) -> p h d", h=4),
                )
            # Q^T / K^T: psum[e(128) x n(512)] accumulated over 8 d-chunks
            def qk_proj(pair):
                for wsb, dst, bias in ((wq, qt, bqs), (wk, kt, bks)):
                    for n4 in range(4):
                        tag = "s1" if n4 % 2 == 0 else "s2"
                        pt = ps.tile([128, 512], F32, tag=tag, name=f"psp{pair}{n4}")
                        for dc in range(DC):
                            nc.tensor.matmul(
                                pt[:, :],
                                wsb[:, dc, pair * 128:(pair + 1) * 128],
                                xt[:, dc, n4 * 512:(n4 + 1) * 512],
                                start=(dc == 0), stop=(dc == DC - 1),
                            )
                        nc.vector.tensor_scalar_add(
                            dst[:, pair, n4 * 512:(n4 + 1) * 512], pt[:, :],
                            bias[:, pair:pair + 1],
                        )
            qk_proj(0)

            # ---- phase 2+3 interleaved by q-half for cross-phase overlap ----
            def attn(pair, q2):
                q0 = q2 * 1024
                if True:
                    pv = [ps.tile([HD + 1, 1024], F32, tag=t, name=f"pv_{t}")
                          for t in ("pv1", "pv2")]
                    for k in range(NB):
                        stiles = [ps.tile([128, 1024], F32, tag=("s1", "s2")[hh],
                                          name=f"st{hh}") for hh in range(2)]
                        # interleave head halves so row-tiled matmuls overlap
                        for half in range(2):
                            for hh in range(2):
                                p0 = hh * 64
                                nc.tensor.matmul(
                                    stiles[hh][:, half * 512:(half + 1) * 512],
                                    kt[p0:p0 + 64, pair, k * 128:(k + 1) * 128],
                                    qt[p0:p0 + 64, pair,
                                       q0 + half * 512:q0 + (half + 1) * 512],
                                    start=True, stop=True,
                                    tile_position=(p0, 0),
                                )
                        for hh in range(2):
                            w = wp.tile([128, 1024], DT_PV, tag=("w1", "w2")[hh],
                                        name=f"w{hh}")
                            nc.scalar.activation(w, stiles[hh], AF.Exp, scale=SCALE)
                            for half in range(2):
                                nc.tensor.matmul(
                                    pv[hh][:, half * 512:(half + 1) * 512],
                                    vp[:, k, 2 * pair + hh, 0:HD + 1],
                                    w[:, half * 512:(half + 1) * 512],
                                    start=(k == 0), stop=(k == NB - 1),
                                )
                    # normalize: attn^T[d, q] / den[q]
                    for hh in range(2):
                        den = dn.tile([1, 1024], F32, tag="den", name="den")
                        rec = dn.tile([1, 1024], F32, tag="rec", name="rec")
                        recr = dn.tile([1, 1024], F32R, tag="recr", name="recr")
                        nc.vector.tensor_copy(den, pv[hh][HD:HD + 1, :])
                        nc.vector.reciprocal_approx_fast(rec, den)
                        nc.vector.tensor_copy(recr, rec)
                        bc = ps.tile([HD, 1024], F32, tag=("s1", "s2")[hh], name="bc")
                        for half in range(2):
                            nc.tensor.matmul(
                                bc[:, half * 512:(half + 1) * 512],
                                ones[:, :],
                                recr[:, half * 512:(half + 1) * 512],
                                start=True, stop=True,
                            )
                        u = up.tile([HD, 1024], F32)
                        nc.vector.tensor_copy(u, pv[hh][0:HD, :])
                        nc.vector.tensor_mul(
                            at[hh * 64:hh * 64 + 64, pair, q0:q0 + 1024], u, bc)

            # ---- output projection for one q-half ----
            def oproj(q2):
                for nb in range(q2 * 8, q2 * 8 + 8):
                    tag = "pv1" if nb % 2 == 0 else "pv2"
                    po = ps.tile([128, 1024], F32, tag=tag, name=f"po{nb}")
                    for half in range(2):
                        for ec in range(2):
                            nc.tensor.matmul(
                                po[:, half * 512:(half + 1) * 512],
                                at[:, ec, nb * 128:(nb + 1) * 128],
                                wo[:, ec, half * 512:(half + 1) * 512],
                                start=(ec == 0), stop=(ec == 1),
                            )
                    ot = op.tile([128, 1024], F32)
                    nc.vector.tensor_copy(ot, po)
                    nc.sync.dma_start(out=out[nb * 128:(nb + 1) * 128, :], in_=ot)

            attn(0, 0)
            qk_proj(1)
            attn(1, 0)
            oproj(0)
            attn(0, 1)
            attn(1, 1)
            oproj(1)
    return nc


_CACHE = {}


def _build():
    if "nc" not in _CACHE:
        nc = bacc.Bacc("TRN2", target_bir_lowering=False, debug=False)
        _emit(nc)
        nc.compile()
        _CACHE["nc"] = nc
    return _CACHE["nc"]


def make_in_maps(x, Wq, bq, Wk, bk, Wv, bv, Wo, bo):
    import ml_dtypes
    f32 = np.float32
    bt = ml_dtypes.bfloat16
    dpr = bt if DT_PR == BF16 else f32
    ones_np = np.ones((128, NB, 4), bt if DT_PV == BF16 else f32)
    xTs = [np.ascontiguousarray(np.asarray(x[b], dtype=f32).T).astype(dpr)
           for b in range(B)]
    in_maps = []
    for c in range(8):
        b, r0 = c // 4, (c % 4) * E
        rows = slice(r0, r0 + E)
        in_maps.append({
            "xT": xTs[b],
            "wqT": np.ascontiguousarray(np.asarray(Wq, f32)[rows].T).astype(dpr),
            "wkT": np.ascontiguousarray(np.asarray(Wk, f32)[rows].T).astype(dpr),
            "wvT": np.ascontiguousarray(np.asarray(Wv, f32)[rows].T).astype(dpr),
            "woT": np.ascontiguousarray(np.asarray(Wo, f32)[:, rows].T),
            "bq2": np.ascontiguousarray(np.asarray(bq, f32)[rows].reshape(2, 128).T),
            "bk2": np.ascontiguousarray(np.asarray(bk, f32)[rows].reshape(2, 128).T),
            "bv1": np.ascontiguousarray(np.asarray(bv, f32)[rows]),
            "vones": ones_np,
            "onesr": np.ones((1, HD), f32),
        })
    return in_maps


def kernel(x, Wq, bq, Wk, bk, Wv, bv, Wo, bo, _spmd_kwargs=None):
    nc = _build()
    in_maps = make_in_maps(x, Wq, bq, Wk, bk, Wv, bv, Wo, bo)
    res = run_bass_kernel_spmd(nc, in_maps, core_ids=list(range(8)),
                               **(_spmd_kwargs or {}))
    parts = np.stack([res.results[c]["out"] for c in range(8)])
    outv = parts.reshape(B, 4, N, D).sum(axis=1) + np.asarray(bo, np.float32)
    if _spmd_kwargs:
        _CACHE["last_results"] = res
    return outv.astype(np.float32)
